# revision 27
# baseline (speedup 1.0000x reference)
"""Trainium2 Bass kernel for SAGAN-style spatial self-attention.

Reference computation (per batch b):
    xf = x[b].reshape(C, N)                    # C=256, N=64*64=4096
    f  = w1 @ xf                               # [32, N]   (query^T)
    g  = w2 @ xf                               # [32, N]   (key)
    V  = (w3 @ xf)^T                           # [N, C]    (value)
    S  = f^T @ g                               # [N, N]
    O  = softmax(S, axis=-1) @ V               # [N, C]
    out[b] = O^T.reshape(C, H, W) + x[b]

Sharding: 8 cores = 4 batches x 2 query-halves. Each core holds its batch's
full xf (keys/values) and computes attention for 2048 query positions. The
host permutes xf columns so each core's 2048 query columns come FIRST —
softmax/PV are permutation-invariant over keys, so no unpermute is needed
and the query projection reads the shared xkv tile directly.

Device algorithm highlights:
  - All inputs arrive host-packed partition-major so every DMA has >=2KB
    contiguous per-partition runs; xkv is chunked 8x across five queue
    rings (sync/gpsimd/tensor/scalar/vector) so the first chunks land
    ~3us after issue and projections start immediately after the prologue.
  - w1/w2 arrive host-replicated 4x along columns, so one [128,512]
    projection matmul yields f (and g) broadcast across all four
    32-partition strips: f_rep[32i+k, :] = f[k, :].
  - S^T m-tiles are computed 4 at a time with row-tiled concurrent matmuls
    (tile_position=(32i,0), K=32): tile i contracts strip i's copy of g
    (m-tile 4g+i, as weights) against strip i's copy of f. The four
    [128,512] outputs land in 4 PSUM banks; one ACT exp drains all 2048
    cols to Pt in bf16. (No max subtraction: |S| <~ 45, exp fits fp32.)
    Each group is emitted AFTER the 16 PV matmuls of its slot so the
    single-buffered 4-bank PSUM slot is free (ACT read done) just in time.
  - V [4096,257] bf16; column 256 is ones so the PV matmul emits the
    softmax denominator for free. Early V-projection copies go through
    ScalarE (idle before the exps start) to keep the DVE off the critical
    path during projections.
  - O chunk: matmul(lhsT=P^T [128m,128n], rhs=V [128m,257]) accumulated
    over 32 m-tiles -> [128n,257]; r = 1/col256; O *= r; add residual
    (f16 x^T) and DMA out in [n, C] layout ([C, n] transpose host-side).

fp16 operands on the PE keep the HAM clock gate open (fp32-mode matmuls do
not register as PE-busy and the PE sticks at 1.2GHz); a short dummy-matmul
bridge warms the PE while the first input DMAs land. A dummy exp right
after the weight DMAs forces the ~1.3us ACT table load off the critical
path.
"""

import sys

sys.path.insert(0, "/opt/trn_rl_repo")

from contextlib import ExitStack

import numpy as np

import concourse.bass as bass
import concourse.tile as tile
from concourse import bacc, mybir
from concourse.bass import ts, ds
from concourse.bass_utils import run_bass_kernel_spmd

F32 = mybir.dt.float32
F16 = mybir.dt.float16
BF16 = mybir.dt.bfloat16

B, C, H, W = 4, 256, 64, 64
N = H * W          # 4096 keys per batch
NQ = N // 2        # 2048 queries per core
CK = 32            # query/key head dim
MT = N // 128      # 32 m-tiles
NCHUNK = NQ // 512  # 4 n-chunks of 512 query cols
EXP = mybir.ActivationFunctionType.Exp


def build_nc():
    nc = bacc.Bacc("TRN2", target_bir_lowering=False, debug=False, num_devices=8)
    xkv_d = nc.dram_tensor("xkv", [128, 8, 2, 512], F16, kind="ExternalInput")
    xqt_d = nc.dram_tensor("xqt", [128, 16, 256], F16, kind="ExternalInput")
    wp_d = nc.dram_tensor("wp", [128, 1024], F16, kind="ExternalInput")
    out_d = nc.dram_tensor("out", [NQ, C], F32, kind="ExternalOutput")

    with tile.TileContext(nc) as tc, ExitStack() as ctx:
        _body(ctx, tc, xkv_d.ap(), xqt_d.ap(), wp_d.ap(), out_d.ap())
    nc.compile()
    return nc


def _body(ctx, tc, xkv_d, xqt_d, wp_d, out_d):
    out_r = out_d.rearrange("(J p) c -> p J c", p=128)
    nc = tc.nc
    singles = ctx.enter_context(tc.tile_pool(name="singles", bufs=1))

    xkv_h = singles.tile([128, 8, 2, 512], F16, tag="xkv_h", name="xkv_h")
    xqt = singles.tile([128, NQ // 128, C], F16, tag="xqt", name="xqt")
    # packed weights: [w1r k0|k1, w2r k0|k1, w3t k0|k1] along the free dim
    wp = singles.tile([128, 1024], F16, tag="wp", name="wp")
    f_rep = singles.tile([128, NQ], F16, tag="f_rep", name="f_rep")
    g_rep = singles.tile([128, N], F16, tag="g_rep", name="g_rep")
    V = singles.tile([128, MT, 260], BF16, tag="V", name="V")
    warm = singles.tile([128, 512], BF16, tag="warm", name="warm")

    # PSUM: st4 = one 4-bank slot for a row-tiled S^T group; op = 1-bank
    # slots for PV accumulators and projection outputs. 4 + 4 = 8 banks.
    stp = ctx.enter_context(tc.tile_pool(name="st_ps", bufs=1, space="PSUM"))
    op = ctx.enter_context(tc.tile_pool(name="o_ps", bufs=4, space="PSUM"))
    ptp = ctx.enter_context(tc.tile_pool(name="pt", bufs=2))
    osbp = ctx.enter_context(tc.tile_pool(name="osb", bufs=2))
    rp = ctx.enter_context(tc.tile_pool(name="r", bufs=3))
    stgp = ctx.enter_context(tc.tile_pool(name="stage", bufs=3))

    # ---- input DMAs, spread across the three queue rings ----
    nc.vector.memset(warm[:], 0.0)
    nc.vector.memset(V[:, :, 256:257], 1.0)
    nc.scalar.dma_start(wp[:], wp_d[:, :])
    # chunk 0 lands in k-halves so the first projection matmul can start
    # as soon as its 128KB is resident
    nc.sync.dma_start(xkv_h[:, 0, 0, :], xkv_d[:, 0, 0, :])
    nc.sync.dma_start(xkv_h[:, 0, 1, :], xkv_d[:, 0, 1, :])
    for ch in (2, 4, 6):
        nc.sync.dma_start(xkv_h[:, ch, :, :], xkv_d[:, ch, :, :])
    for ch in (1, 3, 5, 7):
        nc.gpsimd.dma_start(xkv_h[:, ch, :, :], xkv_d[:, ch, :, :])

    # force the ACT spline-table load now, off the critical path
    dummy = rp.tile([128, 1], F32, tag="r", name="dummy")
    nc.scalar.activation(dummy[:], warm[:, 0:1], EXP)
    nc.scalar.dma_start(xqt[:], xqt_d[:, :, :])

    # HAM warmup bridge: keep the PE streaming while the first input DMAs
    # land so the projection phase starts at 2.4GHz.
    wps = stp.tile([128, 4, 512], F32, tag="st", name="wps")
    for i in range(6):
        nc.tensor.matmul(wps[:, i % 4, :], warm[:, 0:128], warm[:],
                         start=True, stop=True)

    Pt = [None, None]
    stage = [None, None]
    posts = []

    def emit_post(item):
        cc, j, o_ps, stg = item
        J = cc * 4 + j
        r = rp.tile([128, 1], F32, tag="r", name="r")
        nc.vector.reciprocal(r[:], o_ps[:, 256:257])
        nc.vector.scalar_tensor_tensor(stg[:, j, :], o_ps[:, 0:256], r[:],
                                       xqt[:, J, :],
                                       mybir.AluOpType.mult,
                                       mybir.AluOpType.add)
        if cc == NCHUNK - 1:
            qe = (nc.sync, nc.scalar, nc.gpsimd, nc.sync)[j]
            qe.dma_start(out_r[:, J, :], stg[:, j, :])
        elif j == 3:
            nc.gpsimd.dma_start(out_r[:, 4 * cc:4 * cc + 4, :], stg[:, :, :])

    def st_group4(c, g):
        # 4 concurrent row-tiled matmuls: tile i computes S^T for m-tile
        # 4g+i from strip i's replicas of g (weights) and f (stream).
        st = stp.tile([128, 4, 512], F32, tag="st", name="st")
        for i in range(4):
            nc.tensor.matmul(st[:, i, :],
                             g_rep[ds(32 * i, 32), ts(4 * g + i, 128)],
                             f_rep[ds(32 * i, 32), ts(c, 512)],
                             start=True, stop=True,
                             tile_position=(32 * i, 0))
        nc.scalar.activation(Pt[c % 2][:, 4 * g:4 * g + 4, :], st[:], EXP)

    # ---- projections, interleaved with chunk 0 of the scores (S^T lags the
    # g-projection by one key-chunk so the PE never waits on the DVE copy) ----
    Pt[0] = ptp.tile([128, MT, 512], BF16, tag="pt", name="pt")
    for ch in range(N // 512):
        gp = op.tile([128, 512], F32, tag="o", name="gp")
        for k in range(2):
            nc.tensor.matmul(gp[:], wp[:, ds(256 + k * 128, 128)],
                             xkv_h[:, ch, k, :],
                             start=(k == 0), stop=(k == 1))
        nc.vector.tensor_copy(g_rep[:, ts(ch, 512)], gp[:])
        if ch < NQ // 512:
            fp = op.tile([128, 512], F32, tag="o", name="fp")
            for k in range(2):
                nc.tensor.matmul(fp[:], wp[:, ds(k * 128, 128)],
                                 xkv_h[:, ch, k, :],
                                 start=(k == 0), stop=(k == 1))
            nc.vector.tensor_copy(f_rep[:, ts(ch, 512)], fp[:])
        for half in range(2):
            vp = op.tile([128, 2, 256], F32, tag="o", name="vp")
            for t in range(2):
                for k in range(2):
                    nc.tensor.matmul(vp[:, t, :],
                                     xkv_h[:, ch, k, ds((2 * half + t) * 128, 128)],
                                     wp[:, ds(512 + k * 256, 256)],
                                     start=(k == 0), stop=(k == 1))
            mt0 = 4 * ch + 2 * half
            nc.vector.tensor_copy(V[:, mt0:mt0 + 2, 0:256], vp[:])
        if ch >= 1:
            st_group4(0, ch - 1)
    st_group4(0, 7)

    # ---- attention chunks 1..NCHUNK, software-pipelined by one chunk ----
    for c in range(1, NCHUNK + 1):
        if c < NCHUNK:
            Pt[c % 2] = ptp.tile([128, MT, 512], BF16, tag="pt", name="pt")
        stage[(c - 1) % 2] = stgp.tile([128, 4, 256], F32, tag="stage", name="stage")
        o_cur = None
        for gidx in range(8):
            j, seg = gidx // 2, gidx % 2
            if seg == 0:
                o_cur = op.tile([128, 257], F32, tag="o", name="o")
            for mm in range(16):
                mt = seg * 16 + mm
                nc.tensor.matmul(o_cur[:], Pt[(c - 1) % 2][:, mt, ts(j, 128)],
                                 V[:, mt, 0:257],
                                 start=(mt == 0), stop=(mt == MT - 1),
                                 skip_group_check=True)
            if c < NCHUNK:
                st_group4(c, gidx)
            if seg == 1:
                posts.append((c - 1, j, o_cur, stage[(c - 1) % 2]))
            # delay each n-tile's post-processing by one PE group so the DVE
            # normalize never stalls the PE stream; the final chunk has no
            # S^T stream left to protect, so flush immediately there
            while len(posts) > (1 if (gidx < 7 and c < NCHUNK) else 0):
                emit_post(posts.pop(0))
    while posts:
        emit_post(posts.pop(0))


_NC_CACHE = None


def _get_nc():
    global _NC_CACHE
    if _NC_CACHE is None:
        _NC_CACHE = build_nc()
    return _NC_CACHE


def make_in_maps(x, w1, w2, w3):
    x = np.ascontiguousarray(x, dtype=np.float32).reshape(B, C, N)
    # partition-major packs: [p, k, c] with p the SBUF partition, all three
    # weights concatenated along the free dim into one 2KB/partition blob
    w1r = np.tile(w1.T.astype(np.float16), (1, 4)).reshape(2, 128, 128)
    w1r = w1r.transpose(1, 0, 2).reshape(128, 256)
    w2r = np.tile(w2.T.astype(np.float16), (1, 4)).reshape(2, 128, 128)
    w2r = w2r.transpose(1, 0, 2).reshape(128, 256)
    w3t = w3.T.astype(np.float16).reshape(2, 128, 256)
    w3t = w3t.transpose(1, 0, 2).reshape(128, 512)
    wp = np.ascontiguousarray(np.concatenate([w1r, w2r, w3t], axis=1))
    in_maps = []
    xh = x.astype(np.float16)
    for core in range(8):
        b, half = core // 2, core % 2
        q = xh[b][:, half * NQ:(half + 1) * NQ]
        o = xh[b][:, (1 - half) * NQ:(2 - half) * NQ]
        xkv = np.concatenate([q, o], axis=1)              # [256, 4096]
        xkv = xkv.reshape(2, 128, 8, 512).transpose(1, 2, 0, 3)
        xqt = q.T.reshape(16, 128, 256).transpose(1, 0, 2)
        in_maps.append({
            "xkv": np.ascontiguousarray(xkv),
            "xqt": np.ascontiguousarray(xqt),
            "wp": wp,
        })
    return in_maps


def assemble(results):
    out = np.empty((B, C, N), dtype=np.float32)
    for core in range(8):
        b, half = core // 2, core % 2
        out[b][:, half * NQ:(half + 1) * NQ] = results[core]["out"].T
    return out.reshape(B, C, H, W)


def kernel(x, w1, w2, w3):
    nc = _get_nc()
    res = run_bass_kernel_spmd(nc, make_in_maps(x, w1, w2, w3),
                               core_ids=list(range(8)))
    return assemble(res.results)


# revision 28
# speedup vs baseline: 1.0106x; 1.0106x over previous
"""Trainium2 Bass kernel for SAGAN-style spatial self-attention.

Reference computation (per batch b):
    xf = x[b].reshape(C, N)                    # C=256, N=64*64=4096
    f  = w1 @ xf                               # [32, N]   (query^T)
    g  = w2 @ xf                               # [32, N]   (key)
    V  = (w3 @ xf)^T                           # [N, C]    (value)
    S  = f^T @ g                               # [N, N]
    O  = softmax(S, axis=-1) @ V               # [N, C]
    out[b] = O^T.reshape(C, H, W) + x[b]

Sharding: 8 cores = 4 batches x 2 query-halves. Each core holds its batch's
full xf (keys/values) and computes attention for 2048 query positions. The
host permutes xf columns so each core's 2048 query columns come FIRST —
softmax/PV are permutation-invariant over keys, so no unpermute is needed
and the query projection reads the shared xkv tile directly.

Device algorithm highlights:
  - All inputs arrive host-packed partition-major so every DMA has >=2KB
    contiguous per-partition runs; xkv is chunked 8x across five queue
    rings (sync/gpsimd/tensor/scalar/vector) so the first chunks land
    ~3us after issue and projections start immediately after the prologue.
  - w1/w2 arrive host-replicated 4x along columns, so one [128,512]
    projection matmul yields f (and g) broadcast across all four
    32-partition strips: f_rep[32i+k, :] = f[k, :].
  - S^T m-tiles are computed 4 at a time with row-tiled concurrent matmuls
    (tile_position=(32i,0), K=32): tile i contracts strip i's copy of g
    (m-tile 4g+i, as weights) against strip i's copy of f. The four
    [128,512] outputs land in 4 PSUM banks; one ACT exp drains all 2048
    cols to Pt in bf16. (No max subtraction: |S| <~ 45, exp fits fp32.)
    Each group is emitted AFTER the 16 PV matmuls of its slot so the
    single-buffered 4-bank PSUM slot is free (ACT read done) just in time.
  - V [4096,257] bf16; column 256 is ones so the PV matmul emits the
    softmax denominator for free. Early V-projection copies go through
    ScalarE (idle before the exps start) to keep the DVE off the critical
    path during projections.
  - O chunk: matmul(lhsT=P^T [128m,128n], rhs=V [128m,257]) accumulated
    over 32 m-tiles -> [128n,257]; r = 1/col256; O *= r; add residual
    (f16 x^T) and DMA out in [n, C] layout ([C, n] transpose host-side).

fp16 operands on the PE keep the HAM clock gate open (fp32-mode matmuls do
not register as PE-busy and the PE sticks at 1.2GHz); a short dummy-matmul
bridge warms the PE while the first input DMAs land. A dummy exp right
after the weight DMAs forces the ~1.3us ACT table load off the critical
path.
"""

import sys

sys.path.insert(0, "/opt/trn_rl_repo")

from contextlib import ExitStack

import numpy as np

import concourse.bass as bass
import concourse.tile as tile
from concourse import bacc, mybir
from concourse.bass import ts, ds
from concourse.bass_utils import run_bass_kernel_spmd

F32 = mybir.dt.float32
F16 = mybir.dt.float16
BF16 = mybir.dt.bfloat16

B, C, H, W = 4, 256, 64, 64
N = H * W          # 4096 keys per batch
NQ = N // 2        # 2048 queries per core
CK = 32            # query/key head dim
MT = N // 128      # 32 m-tiles
NCHUNK = NQ // 512  # 4 n-chunks of 512 query cols
EXP = mybir.ActivationFunctionType.Exp


def build_nc():
    nc = bacc.Bacc("TRN2", target_bir_lowering=False, debug=False, num_devices=8)
    xkv_d = nc.dram_tensor("xkv", [128, 8, 2, 512], F16, kind="ExternalInput")
    xqt_d = nc.dram_tensor("xqt", [128, 16, 256], F16, kind="ExternalInput")
    wp_d = nc.dram_tensor("wp", [128, 1024], F16, kind="ExternalInput")
    out_d = nc.dram_tensor("out", [NQ, C], F32, kind="ExternalOutput")

    with tile.TileContext(nc) as tc, ExitStack() as ctx:
        _body(ctx, tc, xkv_d.ap(), xqt_d.ap(), wp_d.ap(), out_d.ap())
    nc.compile()
    return nc


def _body(ctx, tc, xkv_d, xqt_d, wp_d, out_d):
    out_r = out_d.rearrange("(J p) c -> p J c", p=128)
    nc = tc.nc
    singles = ctx.enter_context(tc.tile_pool(name="singles", bufs=1))

    xkv_h = singles.tile([128, 8, 2, 512], F16, tag="xkv_h", name="xkv_h")
    xqt = singles.tile([128, NQ // 128, C], F16, tag="xqt", name="xqt")
    # packed weights: [w1r k0|k1, w2r k0|k1, w3t k0|k1] along the free dim
    wp = singles.tile([128, 1024], F16, tag="wp", name="wp")
    f_rep = singles.tile([128, NQ], F16, tag="f_rep", name="f_rep")
    g_rep = singles.tile([128, N], F16, tag="g_rep", name="g_rep")
    V = singles.tile([128, MT, 260], BF16, tag="V", name="V")
    warm = singles.tile([128, 512], BF16, tag="warm", name="warm")

    # PSUM: st4 = one 4-bank slot for a row-tiled S^T group; op = 1-bank
    # slots for PV accumulators and projection outputs. 4 + 4 = 8 banks.
    stp = ctx.enter_context(tc.tile_pool(name="st_ps", bufs=1, space="PSUM"))
    op = ctx.enter_context(tc.tile_pool(name="o_ps", bufs=4, space="PSUM"))
    ptp = ctx.enter_context(tc.tile_pool(name="pt", bufs=2))
    osbp = ctx.enter_context(tc.tile_pool(name="osb", bufs=2))
    rp = ctx.enter_context(tc.tile_pool(name="r", bufs=3))
    stgp = ctx.enter_context(tc.tile_pool(name="stage", bufs=3))

    # ---- input DMAs, spread across the three queue rings ----
    nc.vector.memset(warm[:], 0.0)
    nc.vector.memset(V[:, :, 256:257], 1.0)
    nc.scalar.dma_start(wp[:], wp_d[:, :])
    for ch in (0, 2, 4, 6):
        nc.sync.dma_start(xkv_h[:, ch, :, :], xkv_d[:, ch, :, :])
    for ch in (1, 3, 5, 7):
        nc.gpsimd.dma_start(xkv_h[:, ch, :, :], xkv_d[:, ch, :, :])

    # force the ACT spline-table load now, off the critical path
    dummy = rp.tile([128, 1], F32, tag="r", name="dummy")
    nc.scalar.activation(dummy[:], warm[:, 0:1], EXP)
    nc.scalar.dma_start(xqt[:], xqt_d[:, :, :])

    # HAM warmup bridge: keep the PE streaming while the first input DMAs
    # land so the projection phase starts at 2.4GHz.
    wps = stp.tile([128, 4, 512], F32, tag="st", name="wps")
    for i in range(6):
        nc.tensor.matmul(wps[:, i % 4, :], warm[:, 0:128], warm[:],
                         start=True, stop=True)

    Pt = [None, None]
    stage = [None, None]
    posts = []

    def emit_post(item):
        cc, j, o_ps, stg = item
        J = cc * 4 + j
        r = rp.tile([128, 1], F32, tag="r", name="r")
        nc.vector.reciprocal(r[:], o_ps[:, 256:257])
        nc.vector.scalar_tensor_tensor(stg[:, j, :], o_ps[:, 0:256], r[:],
                                       xqt[:, J, :],
                                       mybir.AluOpType.mult,
                                       mybir.AluOpType.add)
        if cc == NCHUNK - 1:
            qe = (nc.sync, nc.scalar, nc.gpsimd, nc.sync)[j]
            qe.dma_start(out_r[:, J, :], stg[:, j, :])
        elif j == 3:
            nc.gpsimd.dma_start(out_r[:, 4 * cc:4 * cc + 4, :], stg[:, :, :])

    def st_group4(c, g):
        # 4 concurrent row-tiled matmuls: tile i computes S^T for m-tile
        # 4g+i from strip i's replicas of g (weights) and f (stream).
        st = stp.tile([128, 4, 512], F32, tag="st", name="st")
        for i in range(4):
            nc.tensor.matmul(st[:, i, :],
                             g_rep[ds(32 * i, 32), ts(4 * g + i, 128)],
                             f_rep[ds(32 * i, 32), ts(c, 512)],
                             start=True, stop=True,
                             tile_position=(32 * i, 0))
        nc.scalar.activation(Pt[c % 2][:, 4 * g:4 * g + 4, :], st[:], EXP)

    # ---- projections, interleaved with chunk 0 of the scores (S^T lags the
    # g-projection by one key-chunk so the PE never waits on the DVE copy) ----
    Pt[0] = ptp.tile([128, MT, 512], BF16, tag="pt", name="pt")
    for ch in range(N // 512):
        gp = op.tile([128, 512], F32, tag="o", name="gp")
        for k in range(2):
            nc.tensor.matmul(gp[:], wp[:, ds(256 + k * 128, 128)],
                             xkv_h[:, ch, k, :],
                             start=(k == 0), stop=(k == 1))
        nc.vector.tensor_copy(g_rep[:, ts(ch, 512)], gp[:])
        if ch < NQ // 512:
            fp = op.tile([128, 512], F32, tag="o", name="fp")
            for k in range(2):
                nc.tensor.matmul(fp[:], wp[:, ds(k * 128, 128)],
                                 xkv_h[:, ch, k, :],
                                 start=(k == 0), stop=(k == 1))
            nc.vector.tensor_copy(f_rep[:, ts(ch, 512)], fp[:])
        for half in range(2):
            vp = op.tile([128, 2, 256], F32, tag="o", name="vp")
            for t in range(2):
                for k in range(2):
                    nc.tensor.matmul(vp[:, t, :],
                                     xkv_h[:, ch, k, ds((2 * half + t) * 128, 128)],
                                     wp[:, ds(512 + k * 256, 256)],
                                     start=(k == 0), stop=(k == 1))
            mt0 = 4 * ch + 2 * half
            nc.vector.tensor_copy(V[:, mt0:mt0 + 2, 0:256], vp[:])
        if ch >= 1:
            st_group4(0, ch - 1)
    st_group4(0, 7)

    # ---- attention chunks 1..NCHUNK, software-pipelined by one chunk ----
    for c in range(1, NCHUNK + 1):
        if c < NCHUNK:
            Pt[c % 2] = ptp.tile([128, MT, 512], BF16, tag="pt", name="pt")
        stage[(c - 1) % 2] = stgp.tile([128, 4, 256], F32, tag="stage", name="stage")
        o_cur = None
        for gidx in range(8):
            j, seg = gidx // 2, gidx % 2
            if seg == 0:
                o_cur = op.tile([128, 257], F32, tag="o", name="o")
            for mm in range(16):
                mt = seg * 16 + mm
                nc.tensor.matmul(o_cur[:], Pt[(c - 1) % 2][:, mt, ts(j, 128)],
                                 V[:, mt, 0:257],
                                 start=(mt == 0), stop=(mt == MT - 1),
                                 skip_group_check=True)
            if c < NCHUNK:
                st_group4(c, gidx)
            if seg == 1:
                posts.append((c - 1, j, o_cur, stage[(c - 1) % 2]))
            # delay each n-tile's post-processing by one PE group so the DVE
            # normalize never stalls the PE stream; the final chunk has no
            # S^T stream left to protect, so flush immediately there
            while len(posts) > (1 if (gidx < 7 and c < NCHUNK) else 0):
                emit_post(posts.pop(0))
    while posts:
        emit_post(posts.pop(0))


_NC_CACHE = None


def _get_nc():
    global _NC_CACHE
    if _NC_CACHE is None:
        _NC_CACHE = build_nc()
    return _NC_CACHE


def make_in_maps(x, w1, w2, w3):
    x = np.ascontiguousarray(x, dtype=np.float32).reshape(B, C, N)
    # partition-major packs: [p, k, c] with p the SBUF partition, all three
    # weights concatenated along the free dim into one 2KB/partition blob
    w1r = np.tile(w1.T.astype(np.float16), (1, 4)).reshape(2, 128, 128)
    w1r = w1r.transpose(1, 0, 2).reshape(128, 256)
    w2r = np.tile(w2.T.astype(np.float16), (1, 4)).reshape(2, 128, 128)
    w2r = w2r.transpose(1, 0, 2).reshape(128, 256)
    w3t = w3.T.astype(np.float16).reshape(2, 128, 256)
    w3t = w3t.transpose(1, 0, 2).reshape(128, 512)
    wp = np.ascontiguousarray(np.concatenate([w1r, w2r, w3t], axis=1))
    in_maps = []
    xh = x.astype(np.float16)
    for core in range(8):
        b, half = core // 2, core % 2
        q = xh[b][:, half * NQ:(half + 1) * NQ]
        o = xh[b][:, (1 - half) * NQ:(2 - half) * NQ]
        xkv = np.concatenate([q, o], axis=1)              # [256, 4096]
        xkv = xkv.reshape(2, 128, 8, 512).transpose(1, 2, 0, 3)
        xqt = q.T.reshape(16, 128, 256).transpose(1, 0, 2)
        in_maps.append({
            "xkv": np.ascontiguousarray(xkv),
            "xqt": np.ascontiguousarray(xqt),
            "wp": wp,
        })
    return in_maps


def assemble(results):
    out = np.empty((B, C, N), dtype=np.float32)
    for core in range(8):
        b, half = core // 2, core % 2
        out[b][:, half * NQ:(half + 1) * NQ] = results[core]["out"].T
    return out.reshape(B, C, H, W)


def kernel(x, w1, w2, w3):
    nc = _get_nc()
    res = run_bass_kernel_spmd(nc, make_in_maps(x, w1, w2, w3),
                               core_ids=list(range(8)))
    return assemble(res.results)
